# revision 13
# baseline (speedup 1.0000x reference)
"""Multi-head causal self-attention (B=2, S=2048, D=2048, H=16, hd=128) on
8 Trainium2 NeuronCores.

Sharding: core c -> (batch b = c // 4, head-group hg = c % 4). Each core
computes 4 heads of one batch element end-to-end (QKV projections, causal
softmax attention, and its partial contribution to the output projection).
The wo input dim is split across head-groups, so each core returns a partial
[S, D] output (bf16); the host sums the 4 head-group partials per batch
element (the "all-reduce" of tensor parallelism, done on host during
unsharding).

Device kernel layout notes (per core):
- Host pre-transposes activations/weights so every matmul operand already has
  its contraction dim on SBUF partitions; no on-chip transposes are needed.
- Scores are computed TRANSPOSED: S^T[k, q] = xk^T.T @ xq^T per 128-k-block,
  so the exp'd tile is directly the moving operand of the attention@V matmul.
- Softmax uses exp(score * 1/sqrt(hd) - 4) with no row-max pass (scores are
  bounded ~|5.5| for these inputs, so exp is safe in fp32), and row sums are
  reduced across k-blocks by a wide-op pairwise tree on the vector engine,
  finished by a single all-ones matmul per (q-chunk, head) group for the
  cross-partition reduction. This keeps the tensor engine's row-sum cost at
  one 512-wide matmul per group instead of one per k-block.
- Projections run in consumer order (K, Q chunk 3, V, Q chunks 1/2/0) so the
  first attention group's operands are ready the moment projections drain.
- All matmul operands are bf16 (fp32 PSUM accumulation); softmax stats fp32.
"""

import math
import sys

sys.path.insert(0, "/opt/trn_rl_repo")

import ml_dtypes
import numpy as np

import concourse.bass as bass
import concourse.mybir as mybir
import concourse.tile as tile
from concourse.vector_clock import ScopedClock

B, S, D = 2, 2048, 2048
HG = 4          # heads per core
HD = 128        # head dim
LJ = HG * HD    # local (per-core) projection width = 512
P = 128
NC = 8
FP32 = mybir.dt.float32
BF16 = mybir.dt.bfloat16
SCALE = 1.0 / math.sqrt(HD)
EBIAS = -4.0    # constant shift inside exp; cancels in softmax


# ---------------------------------------------------------------------------
# Workaround for walrus "Too many sync wait commands" on the TileContext
# kernel-tail drain: this walrus build accepts very few sync waits per
# instruction, but the tail drain carries one wait per logical processor
# used. Split the waits across preceding SP nops (SP executes in order, so
# the drain still runs after every wait is satisfied).
def _patched_drain_and_barrier(self, tick_clock, wait_clock):
    carrier = self.nc.sync.nop(nofuse=True, hint="tail_drain_waits")
    wait_clock.add_sem_waits(
        carrier.ins, ScopedClock({None: tick_clock.global_clock})
    )
    si = carrier.ins.sync_info
    waits = list(si.on_wait) if si is not None and si.on_wait else []
    updates = list(si.on_update) if si is not None and si.on_update else []
    # engine-completion waits are implied by the all-engine barrier below
    # (engines execute in order); only DMA queue completion needs the drain
    dma_waits = [w for w in waits if "DMA" in (w.ant_name or "")]
    if dma_waits:
        waits = dma_waits
    if len(waits) > 1:
        carrier.ins.sync_info = mybir.SyncInfo(on_wait=waits[:1], on_update=[])
        for i in range(1, len(waits)):
            extra = self.nc.sync.nop(nofuse=True, hint=f"tail_drain_waits_{i}")
            extra.ins.sync_info = mybir.SyncInfo(
                on_wait=waits[i : i + 1],
                on_update=updates if i == len(waits) - 1 else [],
            )
    self.nc.sync.drain()

    self.nc.all_engine_barrier()
    assert self.sems is not None
    popped = self.nc._tile_sem_poison_stack.pop()
    assert popped is self._sem_poison
    self.nc.clear_and_free_semaphores(list(self.sems.allocated().values()))
    self.nc.all_engine_barrier()


tile.TileContext._drain_and_barrier = _patched_drain_and_barrier


def _split_sync_waits(nc: bass.Bass) -> None:
    """This walrus build accepts only ONE sync wait per instruction (any
    class). Move extra waits onto dedicated same-engine NOPs emitted just
    before the instruction — the engine stream is in-order, so blocking at
    the NOP is equivalent to blocking at the instruction itself."""
    uid = 0
    for fn in nc.m.functions:
        for bb in fn.blocks:
            new_insts = []
            for inst in bb.instructions:
                si = inst.sync_info
                waits = list(si.on_wait) if si is not None and si.on_wait else []
                if len(waits) > 1:
                    for w in waits[:-1]:
                        nop = mybir.InstNoOp(
                            name=f"WSPLIT-{uid}", ins=[], outs=[]
                        )
                        uid += 1
                        nop.engine = inst.engine
                        nop.sync_info = mybir.SyncInfo(
                            on_wait=[w], on_update=[]
                        )
                        new_insts.append(nop)
                    inst.sync_info = mybir.SyncInfo(
                        on_wait=[waits[-1]],
                        on_update=list(si.on_update) if si.on_update else [],
                    )
                new_insts.append(inst)
            bb.instructions = new_insts


# ---------------------------------------------------------------------------


def build_bass() -> bass.Bass:
    nc = bass.Bass()
    xq_t = nc.dram_tensor("xq_t", [D, S], BF16, kind="ExternalInput")
    xk_t = nc.dram_tensor("xk_t", [D, S], BF16, kind="ExternalInput")
    xv_t = nc.dram_tensor("xv_t", [D, S], BF16, kind="ExternalInput")
    wq_t = nc.dram_tensor("wq_t", [D, LJ], BF16, kind="ExternalInput")
    wk_t = nc.dram_tensor("wk_t", [D, LJ], BF16, kind="ExternalInput")
    wv_t = nc.dram_tensor("wv_t", [D, LJ], BF16, kind="ExternalInput")
    wo_t = nc.dram_tensor("wo_t", [LJ, D], BF16, kind="ExternalInput")
    mask = nc.dram_tensor("mask", [P, P], BF16, kind="ExternalInput")
    y = nc.dram_tensor("y", [S, D], BF16, kind="ExternalOutput")

    Exp = mybir.ActivationFunctionType.Exp
    Ln = mybir.ActivationFunctionType.Ln
    MUL = mybir.AluOpType.mult
    ADD = mybir.AluOpType.add

    with tile.TileContext(nc) as tc:
        with (
            tc.tile_pool(name="weights", bufs=1) as wpool,
            tc.tile_pool(name="acts", bufs=1) as apool,
        ):
            wo_sb = wpool.tile([P, 4, D], BF16, tag="wo")
            mask_sb = wpool.tile([P, P], BF16, tag="mask")
            ones_sb = wpool.tile([P, P], BF16, tag="ones")
            ebias_sb = wpool.tile([P, 1], FP32, tag="ebias")
            # [d, head, s] transposed projected activations
            xqT_sb = apool.tile([P, HG, S], BF16, tag="xqT")
            xkT_sb = apool.tile([P, HG, S], BF16, tag="xkT")
            # [k within block, k-block, 4 heads x dv] natural-layout V
            xv_sb = apool.tile([P, 16, LJ], BF16, tag="xv")
            # [dv, head, s] transposed attention output (= wo lhsT blocks)
            oT_sb = apool.tile([P, HG, S], BF16, tag="oT")

            nc.vector.memset(ones_sb[:], 1.0)
            nc.vector.memset(ebias_sb[:], EBIAS)
            # PE warmup: dependency-free matmuls fill the tensor engine while
            # the first input DMAs are in flight, and push the HAM activity
            # monitor to full clock before real work begins.
            warm_in = wpool.tile([P, 512], BF16, tag="warm")
            nc.vector.memset(warm_in[:], 1.0)
            with tc.tile_pool(name="warmps", bufs=2, space="PSUM") as warmps:
                # two alternating PSUM tiles keep the warmup pipelined
                # (back-to-back writes to one tile serialize the PE)
                wpss = [
                    warmps.tile([P, 512], FP32, tag="warmps", name=f"w{i}")
                    for i in range(2)
                ]
                for i in range(14):
                    nc.tensor.matmul(
                        wpss[i % 2][:], lhsT=ones_sb[:], rhs=warm_in[:],
                        start=True, stop=True,
                    )

            # ---- Phase 1: projections (weights DMA'd just-in-time so the
            # first matmul only waits for wk + the first input chunk) ----
            with (
                tc.tile_pool(name="qkvw", bufs=1) as qkvw_pool,
                tc.tile_pool(name="xin", bufs=3) as xin_pool,
                tc.tile_pool(name="ppsum", bufs=8, space="PSUM") as ppsum,
            ):
                wq_sb = qkvw_pool.tile([P, 16, LJ], BF16, tag="wq")
                wk_sb = qkvw_pool.tile([P, 16, LJ], BF16, tag="wk")
                wv_sb = qkvw_pool.tile([P, 16, LJ], BF16, tag="wv")
                # Weight and input DMAs are split into halves spread over
                # both DGE rings (sync = hardware DGE, spins up ~4us before
                # the gpsimd software ring), and the contraction (ic) loop
                # is OUTER with 4 held PSUM groups, so the first matmuls
                # only wait for the first half of wk + xk chunk 0.
                def qdma(dst_sb, src_ap, flip):
                    eng = nc.gpsimd if flip else nc.sync
                    eng.dma_start(
                        out=dst_sb[:],
                        in_=src_ap.rearrange("(c p) o -> p c o", p=P),
                    )

                def qdma_interleaved(wsb, wdram, xin, src_sc0, first=False):
                    # halves of the weight and of the first input chunk
                    # alternate across the two rings so the leading matmuls'
                    # operands land first
                    if first:
                        # very first chunk of the kernel: the hardware
                        # (sync) ring spins up ~6us before the software
                        # (gpsimd) ring (~13us), so it carries ic 0..7 of
                        # BOTH operands as quarter DMAs in consumption
                        # order, while the software ring carries ic 8..15
                        # (needed ~19us in, right when it can deliver).
                        for quarter in (0, 1):
                            r0, r1 = quarter * 512, (quarter + 1) * 512
                            for dst_sb, src in ((wsb, wdram), (xin, src_sc0)):
                                nc.sync.dma_start(
                                    out=dst_sb[:, quarter * 4 : quarter * 4 + 4, :],
                                    in_=src[r0:r1, :].rearrange(
                                        "(c p) o -> p c o", p=P
                                    ),
                                )
                        for dst_sb, src in ((wsb, wdram), (xin, src_sc0)):
                            nc.gpsimd.dma_start(
                                out=dst_sb[:, 8:16, :],
                                in_=src[1024:2048, :].rearrange(
                                    "(c p) o -> p c o", p=P
                                ),
                            )
                        return
                    for half in range(2):
                        we = nc.sync if half == 0 else nc.gpsimd
                        xe = nc.gpsimd if half == 0 else nc.sync
                        we.dma_start(
                            out=wsb[:, half * 8 : (half + 1) * 8, :],
                            in_=wdram[
                                half * 1024 : (half + 1) * 1024, :
                            ].rearrange("(c p) o -> p c o", p=P),
                        )
                        xe.dma_start(
                            out=xin[:, half * 8 : (half + 1) * 8, :],
                            in_=src_sc0[
                                half * 1024 : (half + 1) * 1024, :
                            ].rearrange("(c p) o -> p c o", p=P),
                        )

                # xq^T[o, s] and xk^T[o, s]: stationary = weight chunk,
                # moving = pre-transposed input chunk. xq^T is pre-scaled by
                # 1/sqrt(hd) at evacuation so the exp needs no scale.
                def qk_chunk(src, wdram, wsb, dst, evac_scale, sc, first_w,
                             first_all=False):
                    xin = xin_pool.tile([P, 16, 512], BF16, tag="xin")
                    if first_w:
                        qdma_interleaved(
                            wsb, wdram, xin,
                            src[:, sc * 512 : (sc + 1) * 512],
                            first=first_all,
                        )
                    else:
                        qdma(xin, src[:, sc * 512 : (sc + 1) * 512], flip=True)
                    ps = [
                        ppsum.tile([P, 512], FP32, tag="pp", name=f"pp{h}")
                        for h in range(HG)
                    ]
                    for ic in range(16):
                        for h in range(HG):
                            nc.tensor.matmul(
                                ps[h][:],
                                lhsT=wsb[:, ic, h * P : (h + 1) * P],
                                rhs=xin[:, ic, :],
                                start=(ic == 0),
                                stop=(ic == 15),
                            )
                    for h in range(HG):
                        out_sl = dst[:, h, sc * 512 : (sc + 1) * 512]
                        if evac_scale is not None:
                            nc.scalar.mul(out_sl, ps[h][:], evac_scale)
                        else:
                            nc.scalar.copy(out=out_sl, in_=ps[h][:])

                # xv natural [s, dv]: stationary = input chunk, moving = wv
                def v_chunk(sc, first_w):
                    xin = xin_pool.tile([P, 16, 512], BF16, tag="xin")
                    if first_w:
                        qdma_interleaved(
                            wv_sb, wv_t, xin, xv_t[:, sc * 512 : (sc + 1) * 512]
                        )
                    else:
                        qdma(xin, xv_t[:, sc * 512 : (sc + 1) * 512], flip=True)
                    ps = [
                        ppsum.tile([P, 512], FP32, tag="pp", name=f"pp{sbl}")
                        for sbl in range(HG)
                    ]
                    for ic in range(16):
                        for sbl in range(4):
                            nc.tensor.matmul(
                                ps[sbl][:],
                                lhsT=xin[:, ic, sbl * P : (sbl + 1) * P],
                                rhs=wv_sb[:, ic, :],
                                start=(ic == 0),
                                stop=(ic == 15),
                            )
                    for sbl in range(4):
                        nc.scalar.copy(
                            out=xv_sb[:, sc * 4 + sbl, :], in_=ps[sbl][:]
                        )

                # consumer order: attention group (3, 0) needs all of xk
                # plus xq chunk 3; its attn@V tail (drained one group later)
                # needs all of xv. xq chunks 1/2/0 are consumed later.
                for sc in range(4):
                    qk_chunk(xk_t, wk_t, wk_sb, xkT_sb, None, sc,
                             first_w=(sc == 0), first_all=(sc == 0))
                qk_chunk(xq_t, wq_t, wq_sb, xqT_sb, SCALE, 3, first_w=True)
                for sc in range(4):
                    v_chunk(sc, first_w=(sc == 0))
                for sc in (1, 2, 0):
                    qk_chunk(xq_t, wq_t, wq_sb, xqT_sb, SCALE, sc,
                             first_w=False)
                qdma(wo_sb, wo_t, flip=False)
                nc.gpsimd.dma_start(out=mask_sb[:], in_=mask[:])

            # ---- Phases 2+3: attention + output projection, software-
            # pipelined: the consumer-side matmuls (attn@V, row-sum finish,
            # wo) of earlier groups are drained between the score/exp pairs
            # of later groups so the tensor engine never waits on the scalar
            # engine's exp chain. Row sums accumulate on GPSIMD as exp'd
            # tiles are produced. ----
            from collections import deque

            pending = deque()

            def drain(n):
                for _ in range(n):
                    if not pending:
                        return
                    pending.popleft()()

            with (
                tc.tile_pool(name="aT", bufs=5) as aT_pool,
                tc.tile_pool(name="ssum", bufs=2) as ssum_pool,
                tc.tile_pool(name="tsum", bufs=1) as tsum_pool,
                tc.tile_pool(name="rec", bufs=2) as rec_pool,
                tc.tile_pool(name="spsum", bufs=2, space="PSUM") as spsum,
                tc.tile_pool(name="opsum", bufs=3, space="PSUM") as opsum,
                tc.tile_pool(name="aux", bufs=1, space="PSUM") as aux_pool,
                tc.tile_pool(name="yrow", bufs=2) as yrow_pool,
            ):
                # interleave the largest (qc=3) groups with small (qc=1)
                # ones to smooth the scalar engine's exp backlog; qc=0 last
                # keeps the serial tail chain short
                groups = [
                    (3, 0), (1, 0), (3, 1), (1, 1),
                    (3, 2), (1, 2), (3, 3), (1, 3),
                    (2, 0), (0, 0), (2, 1), (0, 1),
                    (2, 2), (0, 2), (2, 3), (0, 3),
                ]
                carry_wo = []

                def tail_thunks(qc, h, aT, ssum_bf):
                    """attn@V matmuls, the row-sum finishing matmul,
                    normalization, and (after the last head of a q-chunk)
                    the wo matmuls, as unit thunks."""
                    q0 = qc * 512
                    nkb = 4 * qc + 4
                    st = {}

                    def pv(kb):
                        def f():
                            if kb == 0:
                                st["o"] = opsum.tile([P, 512], FP32, tag="oo", name="ps_o")
                            m = kb - 4 * qc
                            lo = m * P if m > 0 else 0
                            nc.tensor.matmul(
                                st["o"][:, lo:512],
                                lhsT=xv_sb[:, kb, h * P : (h + 1) * P],
                                rhs=aT[:, kb, lo:512],
                                start=(kb == 0),
                                stop=(kb == nkb - 1),
                            )

                        return f

                    def summ():
                        # cross-partition reduction of the gpsimd-accumulated
                        # per-partition sums: one all-ones matmul per group
                        st["m"] = aux_pool.tile(
                            [P, 512], FP32, tag="aux", name="ps_m"
                        )
                        nc.tensor.matmul(
                            st["m"][:],
                            lhsT=ones_sb[:],
                            rhs=ssum_bf[:],
                            start=True,
                            stop=True,
                        )

                    def fin():
                        # 1/sum = exp(-ln(sum)): both funcs live in the same
                        # ACT table as the softmax exp, so no table reloads,
                        # and it is ~4x faster than the DVE reciprocal.
                        lnm = rec_pool.tile([P, 512], FP32, tag="lnm", name="lnm")
                        nc.scalar.activation(
                            out=lnm[:], in_=st["m"][:], func=Ln
                        )
                        rec = rec_pool.tile([P, 512], FP32, tag="rec", name="rec")
                        nc.scalar.activation(
                            out=rec[:], in_=lnm[:], func=Exp, scale=-1.0
                        )
                        nc.vector.tensor_tensor(
                            out=oT_sb[:, h, q0 : q0 + 512],
                            in0=st["o"][:],
                            in1=rec[:],
                            op=MUL,
                        )

                    thunks = [pv(kb) for kb in range(nkb)]
                    thunks.append(summ)
                    thunks.append(fin)
                    wo_thunks = []

                    if h == HG - 1:
                        # wo for this q-chunk's 4 row blocks
                        for sbl in range(4):
                            sb = qc * 4 + sbl
                            yst = {}

                            def mkrow(sb=sb, yst=yst):
                                def f():
                                    yst["row"] = yrow_pool.tile(
                                        [P, D], BF16, tag="yrow", name="yr"
                                    )

                                return f

                            wo_thunks.append(mkrow())
                            for oc in range(4):

                                def wo_mm(sb=sb, oc=oc, yst=yst):
                                    def f():
                                        ps_y = opsum.tile(
                                            [P, 512], FP32, tag="oo",
                                            name="ps_y",
                                        )
                                        for jc in range(4):
                                            nc.tensor.matmul(
                                                ps_y[:],
                                                lhsT=oT_sb[
                                                    :, jc, sb * P : (sb + 1) * P
                                                ],
                                                rhs=wo_sb[
                                                    :, jc, oc * 512 : (oc + 1) * 512
                                                ],
                                                start=(jc == 0),
                                                stop=(jc == 3),
                                            )
                                        # alternate the evacuation between
                                        # the scalar and vector engines:
                                        # both are ~90% loaded during the
                                        # attention phase, so neither can
                                        # absorb all 64 copies alone
                                        out_sl = yst["row"][
                                            :, oc * 512 : (oc + 1) * 512
                                        ]
                                        if oc % 2 == 0:
                                            nc.scalar.copy(
                                                out=out_sl, in_=ps_y[:]
                                            )
                                        else:
                                            nc.vector.tensor_copy(
                                                out=out_sl, in_=ps_y[:]
                                            )

                                    return f

                                wo_thunks.append(wo_mm())

                            def ydma(sb=sb, yst=yst):
                                def f():
                                    eng = (
                                        nc.sync if sb % 2 == 0 else nc.gpsimd
                                    )
                                    eng.dma_start(
                                        out=y[sb * P : (sb + 1) * P, :],
                                        in_=yst["row"][:],
                                    )

                                return f

                            wo_thunks.append(ydma())
                    return thunks, wo_thunks

                def emit_rowsum(aT, nkb, ssum_bf):
                    """Pairwise-tree sum of the group's exp'd [128,512] tiles
                    on the vector engine, using wide contiguous views (the
                    aT tile is [p, kb, 512] with kb contiguous in free dim).
                    bf16 partials: the same-sign rounding errors of the tree
                    cancel to ~0.1% on the final sum, well inside budget."""

                    def flat(a, b):
                        return aT[:, a:b, :].rearrange("p a b -> p (a b)")

                    def halve_chain(src, width, dst512):
                        while width > 1024:
                            nxt = tsum_pool.tile(
                                [P, width // 2], BF16, tag=f"ts{width // 2}"
                            )
                            nc.vector.tensor_tensor(
                                out=nxt[:],
                                in0=src[:, : width // 2],
                                in1=src[:, width // 2 : width],
                                op=ADD,
                            )
                            src = nxt
                            width //= 2
                        nc.vector.tensor_tensor(
                            out=dst512[:],
                            in0=src[:, :512],
                            in1=src[:, 512:1024],
                            op=ADD,
                        )

                    if nkb in (4, 8, 16):
                        w = nkb * 512 // 2
                        first = tsum_pool.tile([P, w], BF16, tag=f"ts{w}f")
                        nc.vector.tensor_tensor(
                            out=first[:],
                            in0=flat(0, nkb // 2),
                            in1=flat(nkb // 2, nkb),
                            op=ADD,
                        )
                        halve_chain(first, w, ssum_bf)
                    else:  # nkb == 12: reduce 8 + 4, then combine
                        r8 = tsum_pool.tile([P, 512], BF16, tag="tsr8")
                        first = tsum_pool.tile([P, 2048], BF16, tag="ts2048f")
                        nc.vector.tensor_tensor(
                            out=first[:], in0=flat(0, 4), in1=flat(4, 8),
                            op=ADD,
                        )
                        halve_chain(first, 2048, r8)
                        r4 = tsum_pool.tile([P, 1024], BF16, tag="tsr4")
                        nc.vector.tensor_tensor(
                            out=r4[:], in0=flat(8, 10), in1=flat(10, 12),
                            op=ADD,
                        )
                        t = tsum_pool.tile([P, 512], BF16, tag="tsr4b")
                        nc.vector.tensor_tensor(
                            out=t[:], in0=r4[:, :512], in1=r4[:, 512:],
                            op=ADD,
                        )
                        nc.vector.tensor_tensor(
                            out=ssum_bf[:], in0=r8[:], in1=t[:], op=ADD
                        )

                for qc, h in groups:
                    q0 = qc * 512
                    nkb = 4 * qc + 4
                    aT = aT_pool.tile([P, 16, 512], BF16, tag="aT")
                    ssum_bf = ssum_pool.tile([P, 512], BF16, tag="ssumbf")
                    # diag tiles are exp'd only on [m*128:512]; zero the
                    # stale leading region so the full-width tree sum is
                    # correct (queries before the block see zero weight)
                    for m in range(1, 4):
                        nc.vector.memset(aT[:, 4 * qc + m, 0 : m * P], 0.0)
                    for pr in range(nkb // 2):
                        ps = spsum.tile([P, 1024], FP32, tag="ss")
                        halves = []
                        for half in range(2):
                            kb = 2 * pr + half
                            m = kb - 4 * qc
                            lo = m * P if m > 0 else 0
                            nc.tensor.matmul(
                                ps[:, half * 512 + lo : (half + 1) * 512],
                                lhsT=xkT_sb[:, h, kb * P : (kb + 1) * P],
                                rhs=xqT_sb[:, h, q0 + lo : q0 + 512],
                                start=True,
                                stop=True,
                            )
                            halves.append((kb, lo))
                        if halves[0][1] == 0 and halves[1][1] == 0:
                            # both halves full width: one paired exp
                            nc.scalar.activation(
                                out=aT[:, 2 * pr : 2 * pr + 2, :].rearrange(
                                    "p a b -> p (a b)"
                                ),
                                in_=ps[:],
                                func=Exp,
                                bias=ebias_sb[:],
                                scale=1.0,
                            )
                        else:
                            for half, (kb, lo) in enumerate(halves):
                                nc.scalar.activation(
                                    out=aT[:, kb, lo:512],
                                    in_=ps[:, half * 512 + lo : (half + 1) * 512],
                                    func=Exp,
                                    bias=ebias_sb[:],
                                    scale=1.0,
                                )
                        for kb, lo in halves:
                            if kb >= 4 * qc:
                                # triangular-mask the diagonal 128-block
                                m = kb - 4 * qc
                                nc.vector.tensor_tensor(
                                    out=aT[:, kb, m * P : (m + 1) * P],
                                    in0=aT[:, kb, m * P : (m + 1) * P],
                                    in1=mask_sb[:],
                                    op=MUL,
                                )
                        drain(5)
                    emit_rowsum(aT, nkb, ssum_bf)
                    tail, wo_thunks = tail_thunks(qc, h, aT, ssum_bf)
                    pending.extend(tail)
                    if carry_wo:
                        pending.extend(carry_wo)
                        carry_wo = []
                    if wo_thunks:
                        carry_wo = wo_thunks
                while pending:
                    pending.popleft()()
                for t in carry_wo:
                    t()
    _split_sync_waits(nc)
    return nc


_NC_CACHE = None


def _get_nc():
    global _NC_CACHE
    if _NC_CACHE is None:
        _NC_CACHE = build_bass()
    return _NC_CACHE


def _make_mask() -> np.ndarray:
    """[128, 128] upper-triangular-inclusive T[r, c] = 1 iff r <= c: pass
    iff k <= q inside the diagonal 128-block (aT layout is [k, q])."""
    return np.triu(np.ones((P, P), dtype=np.float32)).astype(ml_dtypes.bfloat16)


def make_in_maps(q, k, v, wq, wk, wv, wo):
    bf = ml_dtypes.bfloat16
    mask = _make_mask()
    in_maps = []
    xt = {}
    for b in range(B):
        xt[b] = tuple(
            np.ascontiguousarray(x[b].T).astype(bf) for x in (q, k, v)
        )
    for c in range(NC):
        b, hg = divmod(c, NC // B)
        js = slice(hg * LJ, (hg + 1) * LJ)
        xq_t, xk_t, xv_t = xt[b]
        in_maps.append(
            {
                "xq_t": xq_t,
                "xk_t": xk_t,
                "xv_t": xv_t,
                "wq_t": np.ascontiguousarray(wq[js, :].T).astype(bf),
                "wk_t": np.ascontiguousarray(wk[js, :].T).astype(bf),
                "wv_t": np.ascontiguousarray(wv[js, :].T).astype(bf),
                "wo_t": np.ascontiguousarray(wo[:, js].T).astype(bf),
                "mask": mask,
            }
        )
    return in_maps


def run_sharded(q, k, v, wq, wk, wv, wo, trace=False, tmpdir=None):
    from concourse.bass_utils import run_bass_kernel_spmd

    nc = _get_nc()
    in_maps = make_in_maps(q, k, v, wq, wk, wv, wo)
    res = run_bass_kernel_spmd(
        nc, in_maps, list(range(NC)), trace=trace, tmpdir=tmpdir
    )
    out = np.zeros((B, S, D), dtype=np.float32)
    for c in range(NC):
        out[c // (NC // B)] += res.results[c]["y"].astype(np.float32)
    return out, res


def kernel(q, k, v, wq, wk, wv, wo):
    q = np.asarray(q, dtype=np.float32)
    k = np.asarray(k, dtype=np.float32)
    v = np.asarray(v, dtype=np.float32)
    wq = np.asarray(wq, dtype=np.float32)
    wk = np.asarray(wk, dtype=np.float32)
    wv = np.asarray(wv, dtype=np.float32)
    wo = np.asarray(wo, dtype=np.float32)
    out, _ = run_sharded(q, k, v, wq, wk, wv, wo)
    return out


# revision 15
# speedup vs baseline: 1.0140x; 1.0140x over previous
"""Multi-head causal self-attention (B=2, S=2048, D=2048, H=16, hd=128) on
8 Trainium2 NeuronCores.

Sharding: core c -> (batch b = c // 4, head-group hg = c % 4). Each core
computes 4 heads of one batch element end-to-end (QKV projections, causal
softmax attention, and its partial contribution to the output projection).
The wo input dim is split across head-groups, so each core returns a partial
[S, D] output (bf16); the host sums the 4 head-group partials per batch
element (the "all-reduce" of tensor parallelism, done on host during
unsharding).

Device kernel layout notes (per core):
- Host pre-transposes activations/weights so every matmul operand already has
  its contraction dim on SBUF partitions; no on-chip transposes are needed.
- Scores are computed TRANSPOSED: S^T[k, q] = xk^T.T @ xq^T per 128-k-block,
  so the exp'd tile is directly the moving operand of the attention@V matmul.
- Softmax uses exp(score * 1/sqrt(hd) - 4) with no row-max pass (scores are
  bounded ~|5.5| for these inputs, so exp is safe in fp32), and row sums are
  reduced across k-blocks by a wide-op pairwise tree on the vector engine,
  finished by a single all-ones matmul per (q-chunk, head) group for the
  cross-partition reduction. This keeps the tensor engine's row-sum cost at
  one 512-wide matmul per group instead of one per k-block.
- Projections run in consumer order (K, Q chunk 3, V, Q chunks 1/2/0) so the
  first attention group's operands are ready the moment projections drain.
- All matmul operands are bf16 (fp32 PSUM accumulation); softmax stats fp32.
"""

import math
import sys

sys.path.insert(0, "/opt/trn_rl_repo")

import ml_dtypes
import numpy as np

import concourse.bass as bass
import concourse.mybir as mybir
import concourse.tile as tile
from concourse.vector_clock import ScopedClock

B, S, D = 2, 2048, 2048
HG = 4          # heads per core
HD = 128        # head dim
LJ = HG * HD    # local (per-core) projection width = 512
P = 128
NC = 8
FP32 = mybir.dt.float32
BF16 = mybir.dt.bfloat16
SCALE = 1.0 / math.sqrt(HD)
EBIAS = -4.0    # constant shift inside exp; cancels in softmax


# ---------------------------------------------------------------------------
# Workaround for walrus "Too many sync wait commands" on the TileContext
# kernel-tail drain: this walrus build accepts very few sync waits per
# instruction, but the tail drain carries one wait per logical processor
# used. Split the waits across preceding SP nops (SP executes in order, so
# the drain still runs after every wait is satisfied).
def _patched_drain_and_barrier(self, tick_clock, wait_clock):
    carrier = self.nc.sync.nop(nofuse=True, hint="tail_drain_waits")
    wait_clock.add_sem_waits(
        carrier.ins, ScopedClock({None: tick_clock.global_clock})
    )
    si = carrier.ins.sync_info
    waits = list(si.on_wait) if si is not None and si.on_wait else []
    updates = list(si.on_update) if si is not None and si.on_update else []
    # engine-completion waits are implied by the all-engine barrier below
    # (engines execute in order); only DMA queue completion needs the drain
    dma_waits = [w for w in waits if "DMA" in (w.ant_name or "")]
    if dma_waits:
        waits = dma_waits
    if len(waits) > 1:
        carrier.ins.sync_info = mybir.SyncInfo(on_wait=waits[:1], on_update=[])
        for i in range(1, len(waits)):
            extra = self.nc.sync.nop(nofuse=True, hint=f"tail_drain_waits_{i}")
            extra.ins.sync_info = mybir.SyncInfo(
                on_wait=waits[i : i + 1],
                on_update=updates if i == len(waits) - 1 else [],
            )
    self.nc.sync.drain()

    self.nc.all_engine_barrier()
    assert self.sems is not None
    popped = self.nc._tile_sem_poison_stack.pop()
    assert popped is self._sem_poison
    self.nc.clear_and_free_semaphores(list(self.sems.allocated().values()))
    self.nc.all_engine_barrier()


tile.TileContext._drain_and_barrier = _patched_drain_and_barrier


def _split_sync_waits(nc: bass.Bass) -> None:
    """This walrus build accepts only ONE sync wait per instruction (any
    class). Move extra waits onto dedicated same-engine NOPs emitted just
    before the instruction — the engine stream is in-order, so blocking at
    the NOP is equivalent to blocking at the instruction itself."""
    uid = 0
    for fn in nc.m.functions:
        for bb in fn.blocks:
            new_insts = []
            for inst in bb.instructions:
                si = inst.sync_info
                waits = list(si.on_wait) if si is not None and si.on_wait else []
                if len(waits) > 1:
                    for w in waits[:-1]:
                        nop = mybir.InstNoOp(
                            name=f"WSPLIT-{uid}", ins=[], outs=[]
                        )
                        uid += 1
                        nop.engine = inst.engine
                        nop.sync_info = mybir.SyncInfo(
                            on_wait=[w], on_update=[]
                        )
                        new_insts.append(nop)
                    inst.sync_info = mybir.SyncInfo(
                        on_wait=[waits[-1]],
                        on_update=list(si.on_update) if si.on_update else [],
                    )
                new_insts.append(inst)
            bb.instructions = new_insts


# ---------------------------------------------------------------------------


def build_bass() -> bass.Bass:
    nc = bass.Bass()
    xq_t = nc.dram_tensor("xq_t", [D, S], BF16, kind="ExternalInput")
    xk_t = nc.dram_tensor("xk_t", [D, S], BF16, kind="ExternalInput")
    xv_t = nc.dram_tensor("xv_t", [D, S], BF16, kind="ExternalInput")
    wq_t = nc.dram_tensor("wq_t", [D, LJ], BF16, kind="ExternalInput")
    wk_t = nc.dram_tensor("wk_t", [D, LJ], BF16, kind="ExternalInput")
    wv_t = nc.dram_tensor("wv_t", [D, LJ], BF16, kind="ExternalInput")
    wo_t = nc.dram_tensor("wo_t", [LJ, D], BF16, kind="ExternalInput")
    mask = nc.dram_tensor("mask", [P, P], BF16, kind="ExternalInput")
    y = nc.dram_tensor("y", [S, D], BF16, kind="ExternalOutput")

    Exp = mybir.ActivationFunctionType.Exp
    Ln = mybir.ActivationFunctionType.Ln
    MUL = mybir.AluOpType.mult
    ADD = mybir.AluOpType.add

    with tile.TileContext(nc) as tc:
        with (
            tc.tile_pool(name="weights", bufs=1) as wpool,
            tc.tile_pool(name="acts", bufs=1) as apool,
        ):
            wo_sb = wpool.tile([P, 4, D], BF16, tag="wo")
            mask_sb = wpool.tile([P, P], BF16, tag="mask")
            ones_sb = wpool.tile([P, P], BF16, tag="ones")
            ebias_sb = wpool.tile([P, 1], FP32, tag="ebias")
            # [d, head, s] transposed projected activations
            xqT_sb = apool.tile([P, HG, S], BF16, tag="xqT")
            xkT_sb = apool.tile([P, HG, S], BF16, tag="xkT")
            # [k within block, k-block, 4 heads x dv] natural-layout V
            xv_sb = apool.tile([P, 16, LJ], BF16, tag="xv")
            # [dv, head, s] transposed attention output (= wo lhsT blocks)
            oT_sb = apool.tile([P, HG, S], BF16, tag="oT")

            nc.vector.memset(ones_sb[:], 1.0)
            nc.vector.memset(ebias_sb[:], EBIAS)
            # PE warmup: dependency-free matmuls fill the tensor engine while
            # the first input DMAs are in flight, and push the HAM activity
            # monitor to full clock before real work begins.
            warm_in = wpool.tile([P, 512], BF16, tag="warm")
            nc.vector.memset(warm_in[:], 1.0)
            with tc.tile_pool(name="warmps", bufs=1, space="PSUM") as warmps:
                wps = warmps.tile([P, 512], FP32, tag="warmps")
                for _ in range(22):
                    nc.tensor.matmul(
                        wps[:], lhsT=ones_sb[:], rhs=warm_in[:],
                        start=True, stop=True,
                    )

            # ---- Phase 1: projections (weights DMA'd just-in-time so the
            # first matmul only waits for wk + the first input chunk) ----
            with (
                tc.tile_pool(name="qkvw", bufs=1) as qkvw_pool,
                tc.tile_pool(name="xin", bufs=3) as xin_pool,
                tc.tile_pool(name="ppsum", bufs=8, space="PSUM") as ppsum,
            ):
                wq_sb = qkvw_pool.tile([P, 16, LJ], BF16, tag="wq")
                wk_sb = qkvw_pool.tile([P, 16, LJ], BF16, tag="wk")
                wv_sb = qkvw_pool.tile([P, 16, LJ], BF16, tag="wv")
                # Weight and input DMAs are split into halves spread over
                # both DGE rings (sync = hardware DGE, spins up ~4us before
                # the gpsimd software ring), and the contraction (ic) loop
                # is OUTER with 4 held PSUM groups, so the first matmuls
                # only wait for the first half of wk + xk chunk 0.
                def qdma(dst_sb, src_ap, flip):
                    eng = nc.gpsimd if flip else nc.sync
                    eng.dma_start(
                        out=dst_sb[:],
                        in_=src_ap.rearrange("(c p) o -> p c o", p=P),
                    )

                def qdma_interleaved(wsb, wdram, xin, src_sc0, first=False):
                    # halves of the weight and of the first input chunk
                    # alternate across the two rings so the leading matmuls'
                    # operands land first
                    if first:
                        # very first chunk of the kernel: the hardware
                        # (sync) ring spins up ~6us before the software
                        # (gpsimd) ring (~13us). Pieces are queued in
                        # consumption order (ic 0..3, 4..7, 8..15), 2MB per
                        # ring, so each piece lands just before the matmuls
                        # that consume it.
                        def piece(eng, dst_sb, src, c0, c1):
                            eng.dma_start(
                                out=dst_sb[:, c0:c1, :],
                                in_=src[c0 * P : c1 * P, :].rearrange(
                                    "(c p) o -> p c o", p=P
                                ),
                            )

                        piece(nc.sync, wsb, wdram, 0, 4)
                        piece(nc.sync, xin, src_sc0, 0, 4)
                        piece(nc.gpsimd, wsb, wdram, 4, 8)
                        piece(nc.gpsimd, xin, src_sc0, 4, 8)
                        piece(nc.sync, wsb, wdram, 8, 16)
                        piece(nc.gpsimd, xin, src_sc0, 8, 16)
                        return
                    for half in range(2):
                        we = nc.sync if half == 0 else nc.gpsimd
                        xe = nc.gpsimd if half == 0 else nc.sync
                        we.dma_start(
                            out=wsb[:, half * 8 : (half + 1) * 8, :],
                            in_=wdram[
                                half * 1024 : (half + 1) * 1024, :
                            ].rearrange("(c p) o -> p c o", p=P),
                        )
                        xe.dma_start(
                            out=xin[:, half * 8 : (half + 1) * 8, :],
                            in_=src_sc0[
                                half * 1024 : (half + 1) * 1024, :
                            ].rearrange("(c p) o -> p c o", p=P),
                        )

                # xq^T[o, s] and xk^T[o, s]: stationary = weight chunk,
                # moving = pre-transposed input chunk. xq^T is pre-scaled by
                # 1/sqrt(hd) at evacuation so the exp needs no scale.
                def qk_chunk(src, wdram, wsb, dst, evac_scale, sc, first_w,
                             first_all=False):
                    xin = xin_pool.tile([P, 16, 512], BF16, tag="xin")
                    if first_w:
                        qdma_interleaved(
                            wsb, wdram, xin,
                            src[:, sc * 512 : (sc + 1) * 512],
                            first=first_all,
                        )
                    else:
                        qdma(xin, src[:, sc * 512 : (sc + 1) * 512], flip=True)
                    ps = [
                        ppsum.tile([P, 512], FP32, tag="pp", name=f"pp{h}")
                        for h in range(HG)
                    ]
                    for ic in range(16):
                        for h in range(HG):
                            nc.tensor.matmul(
                                ps[h][:],
                                lhsT=wsb[:, ic, h * P : (h + 1) * P],
                                rhs=xin[:, ic, :],
                                start=(ic == 0),
                                stop=(ic == 15),
                            )
                    for h in range(HG):
                        out_sl = dst[:, h, sc * 512 : (sc + 1) * 512]
                        if evac_scale is not None:
                            nc.scalar.mul(out_sl, ps[h][:], evac_scale)
                        else:
                            nc.scalar.copy(out=out_sl, in_=ps[h][:])

                # xv natural [s, dv]: stationary = input chunk, moving = wv
                def v_chunk(sc, first_w):
                    xin = xin_pool.tile([P, 16, 512], BF16, tag="xin")
                    if first_w:
                        qdma_interleaved(
                            wv_sb, wv_t, xin, xv_t[:, sc * 512 : (sc + 1) * 512]
                        )
                    else:
                        qdma(xin, xv_t[:, sc * 512 : (sc + 1) * 512], flip=True)
                    ps = [
                        ppsum.tile([P, 512], FP32, tag="pp", name=f"pp{sbl}")
                        for sbl in range(HG)
                    ]
                    for ic in range(16):
                        for sbl in range(4):
                            nc.tensor.matmul(
                                ps[sbl][:],
                                lhsT=xin[:, ic, sbl * P : (sbl + 1) * P],
                                rhs=wv_sb[:, ic, :],
                                start=(ic == 0),
                                stop=(ic == 15),
                            )
                    for sbl in range(4):
                        nc.scalar.copy(
                            out=xv_sb[:, sc * 4 + sbl, :], in_=ps[sbl][:]
                        )

                # consumer order: attention group (3, 0) needs all of xk
                # plus xq chunk 3; its attn@V tail (drained one group later)
                # needs all of xv. xq chunks 1/2/0 are consumed later.
                for sc in range(4):
                    qk_chunk(xk_t, wk_t, wk_sb, xkT_sb, None, sc,
                             first_w=(sc == 0), first_all=(sc == 0))
                qk_chunk(xq_t, wq_t, wq_sb, xqT_sb, SCALE, 3, first_w=True)
                for sc in range(4):
                    v_chunk(sc, first_w=(sc == 0))
                for sc in (1, 2, 0):
                    qk_chunk(xq_t, wq_t, wq_sb, xqT_sb, SCALE, sc,
                             first_w=False)
                qdma(wo_sb, wo_t, flip=False)
                nc.gpsimd.dma_start(out=mask_sb[:], in_=mask[:])

            # ---- Phases 2+3: attention + output projection, software-
            # pipelined: the consumer-side matmuls (attn@V, row-sum finish,
            # wo) of earlier groups are drained between the score/exp pairs
            # of later groups so the tensor engine never waits on the scalar
            # engine's exp chain. Row sums accumulate on GPSIMD as exp'd
            # tiles are produced. ----
            from collections import deque

            pending = deque()

            def drain(n):
                for _ in range(n):
                    if not pending:
                        return
                    pending.popleft()()

            with (
                tc.tile_pool(name="aT", bufs=5) as aT_pool,
                tc.tile_pool(name="ssum", bufs=2) as ssum_pool,
                tc.tile_pool(name="tsum", bufs=1) as tsum_pool,
                tc.tile_pool(name="rec", bufs=2) as rec_pool,
                tc.tile_pool(name="spsum", bufs=2, space="PSUM") as spsum,
                tc.tile_pool(name="opsum", bufs=3, space="PSUM") as opsum,
                tc.tile_pool(name="aux", bufs=1, space="PSUM") as aux_pool,
                tc.tile_pool(name="yrow", bufs=2) as yrow_pool,
            ):
                # interleave the largest (qc=3) groups with small (qc=1)
                # ones to smooth the scalar engine's exp backlog; qc=0 last
                # keeps the serial tail chain short
                groups = [
                    (3, 0), (1, 0), (3, 1), (1, 1),
                    (3, 2), (1, 2), (3, 3), (1, 3),
                    (2, 0), (0, 0), (2, 1), (0, 1),
                    (2, 2), (0, 2), (2, 3), (0, 3),
                ]
                carry_wo = []

                def tail_thunks(qc, h, aT, ssum_bf):
                    """attn@V matmuls, the row-sum finishing matmul,
                    normalization, and (after the last head of a q-chunk)
                    the wo matmuls, as unit thunks."""
                    q0 = qc * 512
                    nkb = 4 * qc + 4
                    st = {}

                    def pv(kb):
                        def f():
                            if kb == 0:
                                st["o"] = opsum.tile([P, 512], FP32, tag="oo", name="ps_o")
                            m = kb - 4 * qc
                            lo = m * P if m > 0 else 0
                            nc.tensor.matmul(
                                st["o"][:, lo:512],
                                lhsT=xv_sb[:, kb, h * P : (h + 1) * P],
                                rhs=aT[:, kb, lo:512],
                                start=(kb == 0),
                                stop=(kb == nkb - 1),
                            )

                        return f

                    def summ():
                        # cross-partition reduction of the gpsimd-accumulated
                        # per-partition sums: one all-ones matmul per group
                        st["m"] = aux_pool.tile(
                            [P, 512], FP32, tag="aux", name="ps_m"
                        )
                        nc.tensor.matmul(
                            st["m"][:],
                            lhsT=ones_sb[:],
                            rhs=ssum_bf[:],
                            start=True,
                            stop=True,
                        )

                    def fin():
                        # 1/sum = exp(-ln(sum)): both funcs live in the same
                        # ACT table as the softmax exp, so no table reloads,
                        # and it is ~4x faster than the DVE reciprocal.
                        lnm = rec_pool.tile([P, 512], FP32, tag="lnm", name="lnm")
                        nc.scalar.activation(
                            out=lnm[:], in_=st["m"][:], func=Ln
                        )
                        rec = rec_pool.tile([P, 512], FP32, tag="rec", name="rec")
                        nc.scalar.activation(
                            out=rec[:], in_=lnm[:], func=Exp, scale=-1.0
                        )
                        nc.vector.tensor_tensor(
                            out=oT_sb[:, h, q0 : q0 + 512],
                            in0=st["o"][:],
                            in1=rec[:],
                            op=MUL,
                        )

                    thunks = [pv(kb) for kb in range(nkb)]
                    thunks.append(summ)
                    thunks.append(fin)
                    wo_thunks = []

                    if h == HG - 1:
                        # wo for this q-chunk's 4 row blocks
                        for sbl in range(4):
                            sb = qc * 4 + sbl
                            yst = {}

                            def mkrow(sb=sb, yst=yst):
                                def f():
                                    yst["row"] = yrow_pool.tile(
                                        [P, D], BF16, tag="yrow", name="yr"
                                    )

                                return f

                            wo_thunks.append(mkrow())
                            for oc in range(4):

                                def wo_mm(sb=sb, oc=oc, yst=yst):
                                    def f():
                                        ps_y = opsum.tile(
                                            [P, 512], FP32, tag="oo",
                                            name="ps_y",
                                        )
                                        for jc in range(4):
                                            nc.tensor.matmul(
                                                ps_y[:],
                                                lhsT=oT_sb[
                                                    :, jc, sb * P : (sb + 1) * P
                                                ],
                                                rhs=wo_sb[
                                                    :, jc, oc * 512 : (oc + 1) * 512
                                                ],
                                                start=(jc == 0),
                                                stop=(jc == 3),
                                            )
                                        # alternate the evacuation between
                                        # the scalar and vector engines:
                                        # both are ~90% loaded during the
                                        # attention phase, so neither can
                                        # absorb all 64 copies alone
                                        out_sl = yst["row"][
                                            :, oc * 512 : (oc + 1) * 512
                                        ]
                                        if oc % 2 == 0:
                                            nc.scalar.copy(
                                                out=out_sl, in_=ps_y[:]
                                            )
                                        else:
                                            nc.vector.tensor_copy(
                                                out=out_sl, in_=ps_y[:]
                                            )

                                    return f

                                wo_thunks.append(wo_mm())

                            def ydma(sb=sb, yst=yst):
                                def f():
                                    eng = (
                                        nc.sync if sb % 2 == 0 else nc.gpsimd
                                    )
                                    eng.dma_start(
                                        out=y[sb * P : (sb + 1) * P, :],
                                        in_=yst["row"][:],
                                    )

                                return f

                            wo_thunks.append(ydma())
                    return thunks, wo_thunks

                def emit_rowsum(aT, nkb, ssum_bf):
                    """Pairwise-tree sum of the group's exp'd [128,512] tiles
                    on the vector engine, using wide contiguous views (the
                    aT tile is [p, kb, 512] with kb contiguous in free dim).
                    bf16 partials: the same-sign rounding errors of the tree
                    cancel to ~0.1% on the final sum, well inside budget."""

                    def flat(a, b):
                        return aT[:, a:b, :].rearrange("p a b -> p (a b)")

                    def halve_chain(src, width, dst512):
                        while width > 1024:
                            nxt = tsum_pool.tile(
                                [P, width // 2], BF16, tag=f"ts{width // 2}"
                            )
                            nc.vector.tensor_tensor(
                                out=nxt[:],
                                in0=src[:, : width // 2],
                                in1=src[:, width // 2 : width],
                                op=ADD,
                            )
                            src = nxt
                            width //= 2
                        nc.vector.tensor_tensor(
                            out=dst512[:],
                            in0=src[:, :512],
                            in1=src[:, 512:1024],
                            op=ADD,
                        )

                    if nkb in (4, 8, 16):
                        w = nkb * 512 // 2
                        first = tsum_pool.tile([P, w], BF16, tag=f"ts{w}f")
                        nc.vector.tensor_tensor(
                            out=first[:],
                            in0=flat(0, nkb // 2),
                            in1=flat(nkb // 2, nkb),
                            op=ADD,
                        )
                        halve_chain(first, w, ssum_bf)
                    else:  # nkb == 12: reduce 8 + 4, then combine
                        r8 = tsum_pool.tile([P, 512], BF16, tag="tsr8")
                        first = tsum_pool.tile([P, 2048], BF16, tag="ts2048f")
                        nc.vector.tensor_tensor(
                            out=first[:], in0=flat(0, 4), in1=flat(4, 8),
                            op=ADD,
                        )
                        halve_chain(first, 2048, r8)
                        r4 = tsum_pool.tile([P, 1024], BF16, tag="tsr4")
                        nc.vector.tensor_tensor(
                            out=r4[:], in0=flat(8, 10), in1=flat(10, 12),
                            op=ADD,
                        )
                        t = tsum_pool.tile([P, 512], BF16, tag="tsr4b")
                        nc.vector.tensor_tensor(
                            out=t[:], in0=r4[:, :512], in1=r4[:, 512:],
                            op=ADD,
                        )
                        nc.vector.tensor_tensor(
                            out=ssum_bf[:], in0=r8[:], in1=t[:], op=ADD
                        )

                for qc, h in groups:
                    q0 = qc * 512
                    nkb = 4 * qc + 4
                    aT = aT_pool.tile([P, 16, 512], BF16, tag="aT")
                    ssum_bf = ssum_pool.tile([P, 512], BF16, tag="ssumbf")
                    # diag tiles are exp'd only on [m*128:512]; zero the
                    # stale leading region so the full-width tree sum is
                    # correct (queries before the block see zero weight)
                    for m in range(1, 4):
                        nc.vector.memset(aT[:, 4 * qc + m, 0 : m * P], 0.0)
                    for pr in range(nkb // 2):
                        ps = spsum.tile([P, 1024], FP32, tag="ss")
                        halves = []
                        for half in range(2):
                            kb = 2 * pr + half
                            m = kb - 4 * qc
                            lo = m * P if m > 0 else 0
                            nc.tensor.matmul(
                                ps[:, half * 512 + lo : (half + 1) * 512],
                                lhsT=xkT_sb[:, h, kb * P : (kb + 1) * P],
                                rhs=xqT_sb[:, h, q0 + lo : q0 + 512],
                                start=True,
                                stop=True,
                            )
                            halves.append((kb, lo))
                        if halves[0][1] == 0 and halves[1][1] == 0:
                            # both halves full width: one paired exp
                            nc.scalar.activation(
                                out=aT[:, 2 * pr : 2 * pr + 2, :].rearrange(
                                    "p a b -> p (a b)"
                                ),
                                in_=ps[:],
                                func=Exp,
                                bias=ebias_sb[:],
                                scale=1.0,
                            )
                        else:
                            for half, (kb, lo) in enumerate(halves):
                                nc.scalar.activation(
                                    out=aT[:, kb, lo:512],
                                    in_=ps[:, half * 512 + lo : (half + 1) * 512],
                                    func=Exp,
                                    bias=ebias_sb[:],
                                    scale=1.0,
                                )
                        for kb, lo in halves:
                            if kb >= 4 * qc:
                                # triangular-mask the diagonal 128-block
                                m = kb - 4 * qc
                                nc.vector.tensor_tensor(
                                    out=aT[:, kb, m * P : (m + 1) * P],
                                    in0=aT[:, kb, m * P : (m + 1) * P],
                                    in1=mask_sb[:],
                                    op=MUL,
                                )
                        drain(5)
                    emit_rowsum(aT, nkb, ssum_bf)
                    tail, wo_thunks = tail_thunks(qc, h, aT, ssum_bf)
                    pending.extend(tail)
                    if carry_wo:
                        pending.extend(carry_wo)
                        carry_wo = []
                    if wo_thunks:
                        carry_wo = wo_thunks
                while pending:
                    pending.popleft()()
                for t in carry_wo:
                    t()
    _split_sync_waits(nc)
    return nc


_NC_CACHE = None


def _get_nc():
    global _NC_CACHE
    if _NC_CACHE is None:
        _NC_CACHE = build_bass()
    return _NC_CACHE


def _make_mask() -> np.ndarray:
    """[128, 128] upper-triangular-inclusive T[r, c] = 1 iff r <= c: pass
    iff k <= q inside the diagonal 128-block (aT layout is [k, q])."""
    return np.triu(np.ones((P, P), dtype=np.float32)).astype(ml_dtypes.bfloat16)


def make_in_maps(q, k, v, wq, wk, wv, wo):
    bf = ml_dtypes.bfloat16
    mask = _make_mask()
    in_maps = []
    xt = {}
    for b in range(B):
        xt[b] = tuple(
            np.ascontiguousarray(x[b].T).astype(bf) for x in (q, k, v)
        )
    for c in range(NC):
        b, hg = divmod(c, NC // B)
        js = slice(hg * LJ, (hg + 1) * LJ)
        xq_t, xk_t, xv_t = xt[b]
        in_maps.append(
            {
                "xq_t": xq_t,
                "xk_t": xk_t,
                "xv_t": xv_t,
                "wq_t": np.ascontiguousarray(wq[js, :].T).astype(bf),
                "wk_t": np.ascontiguousarray(wk[js, :].T).astype(bf),
                "wv_t": np.ascontiguousarray(wv[js, :].T).astype(bf),
                "wo_t": np.ascontiguousarray(wo[:, js].T).astype(bf),
                "mask": mask,
            }
        )
    return in_maps


def run_sharded(q, k, v, wq, wk, wv, wo, trace=False, tmpdir=None):
    from concourse.bass_utils import run_bass_kernel_spmd

    nc = _get_nc()
    in_maps = make_in_maps(q, k, v, wq, wk, wv, wo)
    res = run_bass_kernel_spmd(
        nc, in_maps, list(range(NC)), trace=trace, tmpdir=tmpdir
    )
    out = np.zeros((B, S, D), dtype=np.float32)
    for c in range(NC):
        out[c // (NC // B)] += res.results[c]["y"].astype(np.float32)
    return out, res


def kernel(q, k, v, wq, wk, wv, wo):
    q = np.asarray(q, dtype=np.float32)
    k = np.asarray(k, dtype=np.float32)
    v = np.asarray(v, dtype=np.float32)
    wq = np.asarray(wq, dtype=np.float32)
    wk = np.asarray(wk, dtype=np.float32)
    wv = np.asarray(wv, dtype=np.float32)
    wo = np.asarray(wo, dtype=np.float32)
    out, _ = run_sharded(q, k, v, wq, wk, wv, wo)
    return out
